# revision 52
# baseline (speedup 1.0000x reference)
"""Trainium2 Bass kernel for nn_Diff_prop_18425409699925 (GNN message passing).

Math (per batch element b, x = local_feat[b] reshaped to [n=1024, c=256]):
  xn   = x / ||x||_row
  A    = xn @ xn^T                      (W_adj has unit diagonal; einsum uses
                                         only diag(W_adj))
  G    = exp(5*A), diagonal zeroed      (the reference's row-max shift cancels
                                         in the row-normalized mean)
  M    = (G @ x) / rowsum(G)
  diff = (x - M) @ W_aff^T + b_aff
  y    = LeakyReLU(LayerNorm(diff), 0.01)

Sharding: data-parallel over batch B=8, one element per NeuronCore, no
collectives.

Single fused pipeline per core (v2 restructure):
  phase 1: row norms via DVE square+accum, rsqrt via fp32 magic+Newton on DVE
           (keeps ScalarE down to ONE activation-table set), xn in bf16,
           xn^T built with DMA-engine xbar transposes (no PSUM, no DVE copies).
  phase 2: per row-block k: A row-block matmul (bf16), exp on ScalarE with
           row-sum accumulation, G diagonal zeroed, and the G @ x product
           accumulated immediately as pyT += X_k^T @ G[k] with X stationary
           (c-major output) so the PE streams 512-col matmuls with hidden
           weight loads and stays warm.
  phase 3: -1/rowsum broadcast via PE rank-1 matmuls, spyT = pyT * (-beta),
           diff[i] = x_i @ W^T - (py*beta)_i @ W^T accumulated in PSUM from
           c-major operands (no D transposes), LayerNorm stats via bn_stats/
           bn_aggr, and a single fused Prelu(scale=rstd, bias=-mu*rstd,
           alpha=0.01) finisher straight out of PSUM.

global_feat and pos are unused by the reference; accepted and ignored.
"""

import os
import sys

import numpy as np

for _p in ("/opt/trn_rl_repo",):
    if os.path.isdir(_p) and _p not in sys.path:
        sys.path.insert(0, _p)

import ml_dtypes

import concourse.bacc as bacc
import concourse.bass as bass
import concourse.tile as tile
from concourse import mybir
from concourse.bass_utils import run_bass_kernel_spmd

B, T, NN, C = 8, 16, 64, 256
N = T * NN            # 1024 nodes per batch element
P = 128               # partitions
NT = N // P           # 8 n-tiles
CT = C // P           # 2 c-tiles
F32 = mybir.dt.float32
BF16 = mybir.dt.bfloat16
F8 = mybir.dt.float8e4
I32 = mybir.dt.int32
TS = bass.ts
DR = mybir.MatmulPerfMode.DoubleRow

LN_EPS = 1e-5
LEAK = 0.01
E5 = 148.4131591025766          # exp(5): diagonal of exp(5*A) pre-zeroing
MAGIC = float(0x5F3759DF)       # fp32 rsqrt seed constant, as a float


def _emit_rsqrt(nc, sb, out, in_, tmp_tags, n):
    """out[:, :n] = 1/sqrt(in_[:, :n]) entirely on DVE (magic seed + 2 Newton
    steps). in_/out fp32 [P, n]. Avoids the ScalarE sqrt table set."""
    mult = mybir.AluOpType.mult
    add = mybir.AluOpType.add
    bypass = mybir.AluOpType.bypass
    f = sb.tile([P, n], F32, tag=tmp_tags[0], name="rsq_f")
    ii = sb.tile([P, n], I32, tag=tmp_tags[1], name="rsq_i")
    a = sb.tile([P, n], F32, tag=tmp_tags[2], name="rsq_a")
    c = sb.tile([P, n], F32, tag=tmp_tags[3], name="rsq_c")
    # seed: y = bitcast(MAGIC - (bitcast_int(s) >> 1)), done in fp32 arithmetic
    nc.vector.tensor_copy(f[:], in_.bitcast(I32))          # int -> float value
    nc.vector.tensor_scalar(out=f[:], in0=f[:], scalar1=-0.5, scalar2=MAGIC,
                            op0=mult, op1=add)
    nc.vector.tensor_copy(ii[:], f[:])                     # float -> int round
    y = ii.bitcast(F32)
    for _ in range(2):
        nc.vector.tensor_tensor(out=a[:], in0=y[:], in1=y[:], op=mult)
        nc.vector.scalar_tensor_tensor(out=a[:], in0=a[:], scalar=-0.5,
                                       in1=in_, op0=mult, op1=mult)
        nc.vector.tensor_scalar(out=c[:], in0=a[:], scalar1=1.5, scalar2=None,
                                op0=add)
        nc.vector.tensor_tensor(out=y[:], in0=y[:], in1=c[:], op=mult)
    nc.vector.tensor_copy(out, y[:])


def _build_program(diag_one, ln_trivial, cfg):
    nc = bacc.Bacc("TRN2", target_bir_lowering=False, debug=False)

    xb_d = nc.declare_dram_parameter("xb", [P, NT, C], F8, isOutput=False)
    xt_d = nc.declare_dram_parameter("xt", [CT, P, N], BF16, isOutput=False)
    wtb_d = nc.declare_dram_parameter("wtb", [CT, P, C], BF16, isOutput=False)
    ng_d = nc.declare_dram_parameter("ng", [P, P], BF16, isOutput=False)
    aux_d = nc.declare_dram_parameter("aux", [P, 8], F32, isOutput=False)
    sel_d = nc.declare_dram_parameter("sel", [NT, NT, P], BF16, isOutput=False)
    y_d = nc.declare_dram_parameter("y", [P, NT, C], BF16, isOutput=True)

    with tile.TileContext(nc) as tc:
        _emit(nc, tc, xb_d, xt_d, wtb_d, ng_d, aux_d, sel_d, y_d,
              diag_one, ln_trivial, cfg)
    nc.finalize()
    return nc


def _emit(nc, tc, xb_d, xt_d, wtb_d, ng_d, aux_d, sel_d, y_d,
          diag_one, ln_trivial, cfg):
    from contextlib import ExitStack

    mult = mybir.AluOpType.mult
    add = mybir.AluOpType.add
    bypass = mybir.AluOpType.bypass
    amax = mybir.AluOpType.max
    subtract = mybir.AluOpType.subtract
    AF = mybir.ActivationFunctionType

    use_prelu = cfg["prelu"]
    n_warm = cfg["warm_mm"]
    xnt_dma = cfg["xnt_dma"]

    with ExitStack() as ctx:
        sb = ctx.enter_context(tc.tile_pool(name="sb", bufs=1))
        ps = ctx.enter_context(tc.tile_pool(name="ps", bufs=1, space="PSUM"))

        # ---------------- SBUF tiles ----------------
        Xb = sb.tile([P, NT, C], F8, tag="Xb", name="Xb")
        xtb = sb.tile([P, CT, N], BF16, tag="xtb", name="xtb")
        wtb = sb.tile([P, CT, C], BF16, tag="wtb", name="wtb")
        xnT = sb.tile([P, CT, N], F8, tag="xnT", name="xnT")
        G8 = sb.tile([P, NT, N], F8, tag="G8", name="G8")
        spyT = [sb.tile([P, N], BF16, tag=f"spyT{k}", name=f"spyT{k}")
                for k in range(CT)]
        bbc = sb.tile([P, N], BF16, tag="bbc", name="bbc")
        ng = sb.tile([P, P], BF16, tag="ng", name="ng")
        aux = sb.tile([P, 8], F32, tag="aux", name="aux")
        sel = sb.tile([NT, NT, P], BF16, tag="sel", name="sel")
        Y = sb.tile([P, NT, C], BF16, tag="Y", name="Y")
        ones1 = sb.tile([1, P], F32, tag="ones1", name="ones1")

        SS = sb.tile([P, NT], F32, tag="SS", name="SS")
        RNO = sb.tile([P, NT], F32, tag="RNO", name="RNO")
        SP = sb.tile([P, NT], F32, tag="SP", name="SP")
        SPs = sb.tile([P, NT], F32, tag="SPs", name="SPs")
        BET = sb.tile([P, NT], F32, tag="BET", name="BET")
        BST = sb.tile([P, NT, 6], F32, tag="BST", name="BST")
        MV = sb.tile([P, NT, 2], F32, tag="MV", name="MV")
        VPE = sb.tile([P, NT], F32, tag="VPE", name="VPE")
        RSTD = sb.tile([P, NT], F32, tag="RSTD", name="RSTD")
        NB = sb.tile([P, NT], F32, tag="NB", name="NB")
        warm1 = sb.tile([P, 1], F32, tag="warm1", name="warm1")

        if not diag_one:
            wdg = sb.tile([P, CT], F32, tag="wdg", name="wdg")
            xnTs = sb.tile([P, CT, N], F8, tag="xnTs", name="xnTs")
        else:
            xnTs = None
        if not ln_trivial:
            gbc = sb.tile([P, C], F32, tag="gbc", name="gbc")
            bebc = sb.tile([P, C], F32, tag="bebc", name="bebc")
            brow = sb.tile([1, C], F32, tag="brow", name="brow")

        # PSUM tags A0/A1 (2 banks each) and PY0/PY1 (2 banks each); tiles are
        # created in first-write order below (generation order per tag).

        # ------- loads: x on sync (HWDGE), weights on gpsimd (SWDGE) so
        # neither the ScalarE nor the x-path queue is blocked ------------
        for q in range(4):
            nc.sync.dma_start(Xb[:, TS(q, 2), :], xb_d[:, TS(q, 2), :])
        nc.gpsimd.dma_start(ng[:], ng_d[:])
        nc.gpsimd.dma_start(aux[:], aux_d[:])
        nc.gpsimd.dma_start(sel[:], sel_d[:])
        nc.gpsimd.dma_start(wtb[:], wtb_d[:].rearrange("c p d -> p c d"))
        nc.gpsimd.dma_start(xtb[:], xt_d[:].rearrange("c p n -> p c n"))
        nc.vector.memset(ones1[:], 1.0)

        # Trigger the single ScalarE table load (exp set) immediately.
        nc.vector.memset(warm1[:], 0.0)
        wscr = sb.tile([P, 1], F32, tag="wscr", name="wscr")
        nc.scalar.activation(wscr[:], warm1[:], AF.Exp)

        if not diag_one:
            # aux[:, 2:4] carries diag(W_adj) rearranged [P, CT]
            nc.vector.tensor_copy(wdg[:], aux[:, 2:4])

        # ---------------- phase 1: norms, xn, xn^T ----------------
        for i in range(NT):
            # split the row-norm squares across DVE and ScalarE
            if i % 2 == 0:
                sqa = sb.tile([P, C], F32, tag="sqa", bufs=2, name="sqa")
                nc.scalar.activation(sqa[:], Xb[:, i, :], AF.Square,
                                     accum_out=SS[:, i:i + 1])
            else:
                sqs = sb.tile([P, C], F32, tag="sqs", bufs=2, name="sqs")
                nc.vector.scalar_tensor_tensor(
                    out=sqs[:], in0=Xb[:, i, :], scalar=1.0, in1=Xb[:, i, :],
                    op0=bypass, op1=mult, accum_out=SS[:, i:i + 1])
            if i == 3:
                _emit_rsqrt(nc, sb, RNO[:, 0:4], SS[:, 0:4],
                            ("rsA", "rsB", "rsC", "rsD"), 4)
        _emit_rsqrt(nc, sb, RNO[:, 4:NT], SS[:, 4:NT],
                    ("rsA", "rsB", "rsC", "rsD"), 4)

        # xn tiles, then PE transposes batched 8-per-PSUM-slot so the
        # PSUM->SBUF copies are one big [128,1024] CAST per c-chunk
        idb = sb.tile([P, P], BF16, tag="idb", name="idb")
        nc.vector.tensor_scalar(out=idb[:], in0=ng[:], scalar1=-1.0,
                                scalar2=1.0, op0=mult, op1=add)
        # Early PE warmup on the raw fp8 x tiles (ready as soon as the DMA
        # lands) so the HAM clock is at 2.4GHz before the transposes/A work.
        # Targets a PY-tag slot: its next generation (pyT) is written late.
        pwarm0 = ps.tile([P, 512], F32, tag="PY0", name="pwarm0")
        for w in range(n_warm):
            nc.tensor.matmul(pwarm0[:, 0:C], Xb[:, 0, 0:P], Xb[:, w % NT, :],
                             start=True, stop=True)

        xn = [sb.tile([P, C], BF16, tag=f"xn{i}", name=f"xn{i}")
              for i in range(NT)]
        ptg = [ps.tile([P, N], BF16, tag=f"A{cc}", name=f"ptg{cc}")
               for cc in range(CT)]
        for i in range(NT):
            nc.vector.tensor_scalar(
                out=xn[i][:], in0=Xb[:, i, :], scalar1=RNO[:, i:i + 1],
                scalar2=None, op0=mult)
            for cc in range(CT):
                nc.tensor.transpose(ptg[cc][:, TS(i, P)], xn[i][:, TS(cc, P)],
                                    idb[:])
            if i % 2 == 1:
                # interleaved keep-warm matmuls (transpose-mode does not
                # count as PE activity for the HAM monitor)
                nc.tensor.matmul(pwarm0[:, 0:C], Xb[:, 0, 0:P],
                                 Xb[:, i, :], start=True, stop=True)
            if i % 4 == 3:
                # per-half copies so the j=0 A matmuls start one half early
                h = i // 4
                for cc in range(CT):
                    nc.vector.tensor_copy(xnT[:, cc, TS(h, 512)],
                                          ptg[cc][:, TS(h, 512)])
        if not diag_one:
            for cc in range(CT):
                nc.vector.tensor_scalar(
                    out=xnTs[:, cc, :], in0=xnT[:, cc, :],
                    scalar1=wdg[:, cc:cc + 1], scalar2=None, op0=mult)
        lhs_xnT = xnTs if not diag_one else xnT

        # ---------------- phase 2: A, exp, pyT accumulation ----------------
        # Software-pipelined emission: the PE queue is in-order, so pyT(k)
        # (which waits on exp(k)) is queued AFTER A(k+2) to avoid
        # head-of-line blocking the A matmul stream on the ScalarE exp chain.
        pyT = [ps.tile([P, N], F32, tag=f"PY{k}", name=f"pyT{k}")
               for k in range(CT)]

        def emit_a_block(k):
            # fp8 DoubleRow: both c-chunks contract in a single matmul
            pa_k = ps.tile([P, N], F32, tag=f"A{k % 2}", name=f"pa{k}")
            for j in range(2):
                nc.tensor.matmul(
                    pa_k[:, TS(j, 512)],
                    lhs_xnT[:, :, TS(k, P)],
                    xnT[:, :, TS(j, 512)],
                    start=True, stop=True, perf_mode=DR)
            return pa_k

        pa_tiles = {0: emit_a_block(0), 1: emit_a_block(1)}
        for k in range(NT):
            nc.scalar.activation(G8[:, k, :], pa_tiles.pop(k)[:], AF.Exp,
                                 scale=5.0, accum_out=SP[:, k:k + 1])
            # zero the diagonal block of G (it is exp(5*||xn_i||^2) ~ e^5)
            nc.vector.tensor_tensor(out=G8[:, k, TS(k, P)],
                                    in0=G8[:, k, TS(k, P)], in1=ng[:], op=mult)
            if k + 2 < NT:
                pa_tiles[k + 2] = emit_a_block(k + 2)
            if k % 2 == 1:
                # fp8 DoubleRow over the (k-1, k) pair of row blocks
                kp = k // 2
                for cc in range(CT):
                    for j in range(2):
                        nc.tensor.matmul(
                            pyT[cc][:, TS(j, 512)],
                            Xb[:, k - 1:k + 1, TS(cc, P)],
                            G8[:, k - 1:k + 1, TS(j, 512)],
                            start=(kp == 0), stop=(kp == NT // 2 - 1),
                            perf_mode=DR)

        # ---------------- phase 3: beta, spyT, diff, LN, prelu -------------
        # s' = rowsum(G) - e^5 (the zeroed diagonal), beta = -1/s'
        nc.vector.tensor_scalar(out=SPs[:], in0=SP[:], scalar1=-E5,
                                scalar2=None, op0=add)
        nc.vector.reciprocal(BET[:], SPs[:])
        nc.vector.tensor_scalar(out=BET[:], in0=BET[:], scalar1=-1.0,
                                scalar2=None, op0=mult)
        # broadcast beta over partitions: one [128,8] transpose, then eight
        # selection matmuls (lhsT = one-hot row pattern) spray each beta
        # block across all 128 partitions.
        pbT = ps.tile([NT, P], F32, tag="A1", name="pbT")
        identf = sb.tile([P, P], F32, tag="identf", name="identf")
        nc.vector.tensor_copy(identf[:], idb[:])
        nc.tensor.transpose(pbT[:], BET[:], identf[:])
        bT8 = sb.tile([NT, P], BF16, tag="bT8", name="bT8")
        nc.vector.tensor_copy(bT8[:], pbT[:])
        pbc = ps.tile([P, N], F32, tag="A0", name="pbc")
        for k in range(NT):
            nc.tensor.matmul(pbc[:, TS(k, P)], sel[:, k, :], bT8[:],
                             start=(k % 4 == 0), stop=(k % 4 == 3))
        for j in range(2):
            # PSUM->SBUF casts on the ScalarE (idle between exp and stats)
            nc.scalar.activation(bbc[:, TS(j, 512)], pbc[:, TS(j, 512)],
                                 AF.Copy)

        # diff tiles: pd[i] = sum_cc (xt_i^T + spyT_i^T) @ wtb_cc.
        # The xt half only needs DMA'd operands, so those matmuls keep the
        # PE busy (and warm) while the DVE builds bbc and spyT.
        pdA = ps.tile([P, N], F32, tag="A1", name="pdA")
        pdB = ps.tile([P, N], F32, tag="A0", name="pdB")
        pd = [pdA[:, TS(i, C)] for i in range(4)] + \
             [pdB[:, TS(i, C)] for i in range(4)]
        half = NT // 2
        # PSUM zero regions are 2KB (a full bank): pd tiles i and i+1 share a
        # bank, so each bank carries ONE accumulation group - start fires on
        # the even tile's first matmul, stop on the odd tile's last.
        for i in range(NT):
            for cc in range(CT):
                nc.tensor.matmul(pd[i], xtb[:, cc, TS(i, P)], wtb[:, cc, :],
                                 start=(i % 2 == 0 and cc == 0), stop=False)
        # spyT = pyT * bbc in 512-col halves interleaved with the matmul
        # pairs; stats stream behind the matmuls. Tiles in ACT_STATS get
        # their LayerNorm sums via ScalarE (Identity/Square + accum) to
        # offload the DVE, which is the phase-3 bottleneck engine.
        ACT_STATS = {2, 5} if ln_trivial else set()
        SU = sb.tile([P, NT], F32, tag="SU", name="SU")
        SQ = sb.tile([P, NT], F32, tag="SQ", name="SQ")

        def emit_stats(i):
            if i in ACT_STATS:
                ssc = sb.tile([P, C], F32, tag="ssc", bufs=2, name="ssc")
                nc.scalar.activation(ssc[:], pd[i], AF.Identity,
                                     accum_out=SU[:, i:i + 1])
                ssq = sb.tile([P, C], F32, tag="ssq", bufs=2, name="ssq")
                nc.scalar.activation(ssq[:], pd[i], AF.Square,
                                     accum_out=SQ[:, i:i + 1])
            else:
                nc.vector.bn_stats(BST[:, i, :], pd[i])
                nc.vector.bn_aggr(MV[:, i, :], BST[:, i, :])

        def emit_finish(h):
            # mu/var for the ScalarE-stats tiles of this half
            for i in sorted(ACT_STATS):
                if i // half != h:
                    continue
                nc.vector.tensor_scalar(out=MV[:, i, 0:1], in0=SU[:, i:i + 1],
                                        scalar1=1.0 / C, scalar2=None,
                                        op0=mult)
                nc.vector.scalar_tensor_tensor(
                    out=VPE[:, i:i + 1], in0=MV[:, i, 0:1], scalar=-1.0,
                    in1=MV[:, i, 0:1], op0=mult, op1=mult)
                nc.vector.scalar_tensor_tensor(
                    out=MV[:, i, 1:2], in0=SQ[:, i:i + 1], scalar=1.0 / C,
                    in1=VPE[:, i:i + 1], op0=mult, op1=add)
            hs = slice(h * half, (h + 1) * half)
            nc.vector.tensor_scalar(out=VPE[:, hs], in0=MV[:, hs, 1],
                                    scalar1=LN_EPS, scalar2=None, op0=add)
            _emit_rsqrt(nc, sb, RSTD[:, hs], VPE[:, hs],
                        ("rsA", "rsB", "rsC", "rsD"), half)
            nc.vector.scalar_tensor_tensor(out=NB[:, hs], in0=MV[:, hs, 0],
                                           scalar=-1.0, in1=RSTD[:, hs],
                                           op0=mult, op1=mult)
            for i in range(h * half, (h + 1) * half):
                if ln_trivial and use_prelu and i % 2 == 0:
                    nc.scalar.activation(Y[:, i, :], pd[i], AF.Prelu,
                                         bias=NB[:, i:i + 1],
                                         scale=RSTD[:, i:i + 1], alpha=LEAK)
                else:
                    t = sb.tile([P, C], BF16, tag="fin", bufs=2, name="fin")
                    nc.vector.tensor_scalar(
                        out=t[:], in0=pd[i], scalar1=RSTD[:, i:i + 1],
                        scalar2=NB[:, i:i + 1], op0=mult, op1=add)
                    if not ln_trivial:
                        u = sb.tile([P, C], F32, tag="fin2", bufs=2,
                                    name="fin2")
                        nc.vector.scalar_tensor_tensor(
                            out=u[:], in0=t[:], scalar=1.0, in1=gbc[:],
                            op0=bypass, op1=mult)
                        nc.vector.scalar_tensor_tensor(
                            out=t[:], in0=u[:], scalar=1.0, in1=bebc[:],
                            op0=bypass, op1=add)
                    nc.vector.scalar_tensor_tensor(
                        out=Y[:, i, :], in0=t[:], scalar=LEAK, in1=t[:],
                        op0=mult, op1=amax)
            nc.sync.dma_start(y_d[:, hs, :], Y[:, hs, :])

        for h in range(2):
            for cc in range(CT):
                nc.vector.tensor_tensor(out=spyT[cc][:, TS(h, 512)],
                                        in0=pyT[cc][:, TS(h, 512)],
                                        in1=bbc[:, TS(h, 512)], op=mult)
            for p in (2 * h, 2 * h + 1):
                for i in (2 * p, 2 * p + 1):
                    for cc in range(CT):
                        last = (i % 2 == 1) and (cc == CT - 1) and ln_trivial
                        nc.tensor.matmul(pd[i], spyT[cc][:, TS(i, P)],
                                         wtb[:, cc, :], start=False, stop=last)
                    if not ln_trivial:
                        nc.tensor.matmul(pd[i], ones1[:], brow[:],
                                         start=False, stop=(i % 2 == 1))
                for i in (2 * p, 2 * p + 1):
                    emit_stats(i)
            if h == 1:
                emit_finish(0)
        emit_finish(1)


_PROGRAM_CACHE = {}
last_results = None


def _cfg():
    return {
        "prelu": bool(int(os.environ.get("KERNEL_PRELU", "1"))),
        "warm_mm": int(os.environ.get("KERNEL_WARM_MM", "8")),
        "xnt_dma": bool(int(os.environ.get("KERNEL_XNT_DMA", "1"))),
    }


def _get_program(diag_one=True, ln_trivial=True):
    cfg = _cfg()
    key = (diag_one, ln_trivial, tuple(sorted(cfg.items())))
    if key not in _PROGRAM_CACHE:
        _PROGRAM_CACHE[key] = _build_program(diag_one, ln_trivial, cfg)
    return _PROGRAM_CACHE[key]


def _prep_inputs(local_feat, W_adj, W_aff, b_aff, ln_gamma, ln_beta):
    x_full = np.asarray(local_feat, np.float32).reshape(B, N, C)
    diag = np.ascontiguousarray(np.diagonal(np.asarray(W_adj, np.float32)))
    diag_one = bool(np.all(diag == 1.0))
    g = np.asarray(ln_gamma, np.float32).ravel()
    be = np.asarray(ln_beta, np.float32).ravel()
    b = np.asarray(b_aff, np.float32).ravel()
    ln_trivial = bool(np.all(g == 1.0) and np.all(be == 0.0)
                      and np.all(b == 0.0))

    bf = ml_dtypes.bfloat16
    # wtb[cc, p, d] = W_aff[d, cc*128 + p]
    wt = np.ascontiguousarray(
        np.asarray(W_aff, np.float32).T.reshape(CT, P, C)).astype(bf)
    ng = (1.0 - np.eye(P, dtype=np.float32)).astype(bf)
    sel = np.broadcast_to(np.eye(NT, dtype=np.float32)[:, :, None],
                          (NT, NT, P)).astype(bf)
    aux = np.zeros((P, 8), np.float32)
    if not diag_one:
        aux[:, 2:4] = diag.reshape(CT, P).T

    f8 = ml_dtypes.float8_e4m3
    in_maps = []
    for bb in range(B):
        x = x_full[bb]
        xb = np.ascontiguousarray(
            x.reshape(NT, P, C).transpose(1, 0, 2)).astype(f8)
        xt = np.ascontiguousarray(
            x.T.reshape(CT, P, N)).astype(bf)
        in_maps.append({"xb": xb, "xt": xt, "wtb": wt, "ng": ng,
                        "aux": aux, "sel": sel})
    return in_maps, diag_one, ln_trivial


def kernel(local_feat, global_feat, pos, W_adj, W_aff, b_aff, ln_gamma,
           ln_beta, **_unused):
    global last_results
    in_maps, diag_one, ln_trivial = _prep_inputs(
        local_feat, W_adj, W_aff, b_aff, ln_gamma, ln_beta)
    nc = _get_program(diag_one, ln_trivial)
    trace = bool(int(os.environ.get("KERNEL_TRACE", "0")))
    res = run_bass_kernel_spmd(nc, in_maps, list(range(B)), trace=trace)
    last_results = res
    out = np.empty((B, N, C), np.float32)
    for bb in range(B):
        yb = np.asarray(res.results[bb]["y"]).astype(np.float32)  # [P, NT, C]
        out[bb] = yb.transpose(1, 0, 2).reshape(N, C)
    return out.reshape(B, T, NN, C)


# revision 53
# speedup vs baseline: 1.0043x; 1.0043x over previous
"""Trainium2 Bass kernel for nn_Diff_prop_18425409699925 (GNN message passing).

Math (per batch element b, x = local_feat[b] reshaped to [n=1024, c=256]):
  xn   = x / ||x||_row
  A    = xn @ xn^T                      (W_adj has unit diagonal; einsum uses
                                         only diag(W_adj))
  G    = exp(5*A), diagonal zeroed      (the reference's row-max shift cancels
                                         in the row-normalized mean)
  M    = (G @ x) / rowsum(G)
  diff = (x - M) @ W_aff^T + b_aff
  y    = LeakyReLU(LayerNorm(diff), 0.01)

Sharding: data-parallel over batch B=8, one element per NeuronCore, no
collectives.

Single fused pipeline per core (v2 restructure):
  phase 1: row norms via DVE square+accum, rsqrt via fp32 magic+Newton on DVE
           (keeps ScalarE down to ONE activation-table set), xn in bf16,
           xn^T built with DMA-engine xbar transposes (no PSUM, no DVE copies).
  phase 2: per row-block k: A row-block matmul (bf16), exp on ScalarE with
           row-sum accumulation, G diagonal zeroed, and the G @ x product
           accumulated immediately as pyT += X_k^T @ G[k] with X stationary
           (c-major output) so the PE streams 512-col matmuls with hidden
           weight loads and stays warm.
  phase 3: -1/rowsum broadcast via PE rank-1 matmuls, spyT = pyT * (-beta),
           diff[i] = x_i @ W^T - (py*beta)_i @ W^T accumulated in PSUM from
           c-major operands (no D transposes), LayerNorm stats via bn_stats/
           bn_aggr, and a single fused Prelu(scale=rstd, bias=-mu*rstd,
           alpha=0.01) finisher straight out of PSUM.

global_feat and pos are unused by the reference; accepted and ignored.
"""

import os
import sys

import numpy as np

for _p in ("/opt/trn_rl_repo",):
    if os.path.isdir(_p) and _p not in sys.path:
        sys.path.insert(0, _p)

import ml_dtypes

import concourse.bacc as bacc
import concourse.bass as bass
import concourse.tile as tile
from concourse import mybir
from concourse.bass_utils import run_bass_kernel_spmd

B, T, NN, C = 8, 16, 64, 256
N = T * NN            # 1024 nodes per batch element
P = 128               # partitions
NT = N // P           # 8 n-tiles
CT = C // P           # 2 c-tiles
F32 = mybir.dt.float32
BF16 = mybir.dt.bfloat16
F8 = mybir.dt.float8e4
I32 = mybir.dt.int32
TS = bass.ts
DR = mybir.MatmulPerfMode.DoubleRow

LN_EPS = 1e-5
LEAK = 0.01
E5 = 148.4131591025766          # exp(5): diagonal of exp(5*A) pre-zeroing
MAGIC = float(0x5F3759DF)       # fp32 rsqrt seed constant, as a float


def _emit_rsqrt(nc, sb, out, in_, tmp_tags, n):
    """out[:, :n] = 1/sqrt(in_[:, :n]) entirely on DVE (magic seed + 2 Newton
    steps). in_/out fp32 [P, n]. Avoids the ScalarE sqrt table set."""
    mult = mybir.AluOpType.mult
    add = mybir.AluOpType.add
    bypass = mybir.AluOpType.bypass
    f = sb.tile([P, n], F32, tag=tmp_tags[0], name="rsq_f")
    ii = sb.tile([P, n], I32, tag=tmp_tags[1], name="rsq_i")
    a = sb.tile([P, n], F32, tag=tmp_tags[2], name="rsq_a")
    c = sb.tile([P, n], F32, tag=tmp_tags[3], name="rsq_c")
    # seed: y = bitcast(MAGIC - (bitcast_int(s) >> 1)), done in fp32 arithmetic
    nc.vector.tensor_copy(f[:], in_.bitcast(I32))          # int -> float value
    nc.vector.tensor_scalar(out=f[:], in0=f[:], scalar1=-0.5, scalar2=MAGIC,
                            op0=mult, op1=add)
    nc.vector.tensor_copy(ii[:], f[:])                     # float -> int round
    y = ii.bitcast(F32)
    for _ in range(2):
        nc.vector.tensor_tensor(out=a[:], in0=y[:], in1=y[:], op=mult)
        nc.vector.scalar_tensor_tensor(out=a[:], in0=a[:], scalar=-0.5,
                                       in1=in_, op0=mult, op1=mult)
        nc.vector.tensor_scalar(out=c[:], in0=a[:], scalar1=1.5, scalar2=None,
                                op0=add)
        nc.vector.tensor_tensor(out=y[:], in0=y[:], in1=c[:], op=mult)
    nc.vector.tensor_copy(out, y[:])


def _build_program(diag_one, ln_trivial, cfg):
    nc = bacc.Bacc("TRN2", target_bir_lowering=False, debug=False)

    xb_d = nc.declare_dram_parameter("xb", [P, NT, C], F8, isOutput=False)
    xt_d = nc.declare_dram_parameter("xt", [CT, P, N], BF16, isOutput=False)
    wtb_d = nc.declare_dram_parameter("wtb", [CT, P, C], BF16, isOutput=False)
    ng_d = nc.declare_dram_parameter("ng", [P, P], BF16, isOutput=False)
    aux_d = nc.declare_dram_parameter("aux", [P, 8], F32, isOutput=False)
    sel_d = nc.declare_dram_parameter("sel", [NT, NT, P], BF16, isOutput=False)
    y_d = nc.declare_dram_parameter("y", [P, NT, C], BF16, isOutput=True)

    with tile.TileContext(nc) as tc:
        _emit(nc, tc, xb_d, xt_d, wtb_d, ng_d, aux_d, sel_d, y_d,
              diag_one, ln_trivial, cfg)
    nc.finalize()
    return nc


def _emit(nc, tc, xb_d, xt_d, wtb_d, ng_d, aux_d, sel_d, y_d,
          diag_one, ln_trivial, cfg):
    from contextlib import ExitStack

    mult = mybir.AluOpType.mult
    add = mybir.AluOpType.add
    bypass = mybir.AluOpType.bypass
    amax = mybir.AluOpType.max
    subtract = mybir.AluOpType.subtract
    AF = mybir.ActivationFunctionType

    use_prelu = cfg["prelu"]
    n_warm = cfg["warm_mm"]
    xnt_dma = cfg["xnt_dma"]

    with ExitStack() as ctx:
        sb = ctx.enter_context(tc.tile_pool(name="sb", bufs=1))
        ps = ctx.enter_context(tc.tile_pool(name="ps", bufs=1, space="PSUM"))

        # ---------------- SBUF tiles ----------------
        Xb = sb.tile([P, NT, C], F8, tag="Xb", name="Xb")
        xtb = sb.tile([P, CT, N], BF16, tag="xtb", name="xtb")
        wtb = sb.tile([P, CT, C], BF16, tag="wtb", name="wtb")
        xnT = sb.tile([P, CT, N], F8, tag="xnT", name="xnT")
        G8 = sb.tile([P, NT, N], F8, tag="G8", name="G8")
        spyT = [sb.tile([P, N], BF16, tag=f"spyT{k}", name=f"spyT{k}")
                for k in range(CT)]
        bbc = sb.tile([P, N], BF16, tag="bbc", name="bbc")
        ng = sb.tile([P, P], BF16, tag="ng", name="ng")
        aux = sb.tile([P, 8], F32, tag="aux", name="aux")
        sel = sb.tile([NT, NT, P], BF16, tag="sel", name="sel")
        Y = sb.tile([P, NT, C], BF16, tag="Y", name="Y")
        ones1 = sb.tile([1, P], F32, tag="ones1", name="ones1")

        SS = sb.tile([P, NT], F32, tag="SS", name="SS")
        RNO = sb.tile([P, NT], F32, tag="RNO", name="RNO")
        SP = sb.tile([P, NT], F32, tag="SP", name="SP")
        SPs = sb.tile([P, NT], F32, tag="SPs", name="SPs")
        BET = sb.tile([P, NT], F32, tag="BET", name="BET")
        BST = sb.tile([P, NT, 6], F32, tag="BST", name="BST")
        MV = sb.tile([P, NT, 2], F32, tag="MV", name="MV")
        VPE = sb.tile([P, NT], F32, tag="VPE", name="VPE")
        RSTD = sb.tile([P, NT], F32, tag="RSTD", name="RSTD")
        NB = sb.tile([P, NT], F32, tag="NB", name="NB")
        warm1 = sb.tile([P, 1], F32, tag="warm1", name="warm1")

        if not diag_one:
            wdg = sb.tile([P, CT], F32, tag="wdg", name="wdg")
            xnTs = sb.tile([P, CT, N], F8, tag="xnTs", name="xnTs")
        else:
            xnTs = None
        if not ln_trivial:
            gbc = sb.tile([P, C], F32, tag="gbc", name="gbc")
            bebc = sb.tile([P, C], F32, tag="bebc", name="bebc")
            brow = sb.tile([1, C], F32, tag="brow", name="brow")

        # PSUM tags A0/A1 (2 banks each) and PY0/PY1 (2 banks each); tiles are
        # created in first-write order below (generation order per tag).

        # ------- loads: x on sync (HWDGE), weights on gpsimd (SWDGE) so
        # neither the ScalarE nor the x-path queue is blocked ------------
        for q in range(4):
            nc.sync.dma_start(Xb[:, TS(q, 2), :], xb_d[:, TS(q, 2), :])
        nc.gpsimd.dma_start(ng[:], ng_d[:])
        nc.gpsimd.dma_start(aux[:], aux_d[:])
        nc.gpsimd.dma_start(sel[:], sel_d[:])
        nc.gpsimd.dma_start(wtb[:], wtb_d[:].rearrange("c p d -> p c d"))
        nc.gpsimd.dma_start(xtb[:], xt_d[:].rearrange("c p n -> p c n"))
        nc.vector.memset(ones1[:], 1.0)

        # Trigger the single ScalarE table load (exp set) immediately.
        nc.vector.memset(warm1[:], 0.0)
        wscr = sb.tile([P, 1], F32, tag="wscr", name="wscr")
        nc.scalar.activation(wscr[:], warm1[:], AF.Exp)

        if not diag_one:
            # aux[:, 2:4] carries diag(W_adj) rearranged [P, CT]
            nc.vector.tensor_copy(wdg[:], aux[:, 2:4])

        # ---------------- phase 1: norms, xn, xn^T ----------------
        for i in range(NT):
            # split the row-norm squares across DVE and ScalarE
            if i % 2 == 0:
                sqa = sb.tile([P, C], F32, tag="sqa", bufs=2, name="sqa")
                nc.scalar.activation(sqa[:], Xb[:, i, :], AF.Square,
                                     accum_out=SS[:, i:i + 1])
            else:
                sqs = sb.tile([P, C], F32, tag="sqs", bufs=2, name="sqs")
                nc.vector.scalar_tensor_tensor(
                    out=sqs[:], in0=Xb[:, i, :], scalar=1.0, in1=Xb[:, i, :],
                    op0=bypass, op1=mult, accum_out=SS[:, i:i + 1])
            if i == 3:
                _emit_rsqrt(nc, sb, RNO[:, 0:4], SS[:, 0:4],
                            ("rsA", "rsB", "rsC", "rsD"), 4)
        _emit_rsqrt(nc, sb, RNO[:, 4:NT], SS[:, 4:NT],
                    ("rsA", "rsB", "rsC", "rsD"), 4)

        # xn tiles, then PE transposes batched 8-per-PSUM-slot so the
        # PSUM->SBUF copies are one big [128,1024] CAST per c-chunk
        idb = sb.tile([P, P], BF16, tag="idb", name="idb")
        nc.vector.tensor_scalar(out=idb[:], in0=ng[:], scalar1=-1.0,
                                scalar2=1.0, op0=mult, op1=add)
        # Early PE warmup on the raw fp8 x tiles (ready as soon as the DMA
        # lands) so the HAM clock is at 2.4GHz before the transposes/A work.
        # Targets a PY-tag slot: its next generation (pyT) is written late.
        pwarm0 = ps.tile([P, 512], F32, tag="PY0", name="pwarm0")
        for w in range(n_warm):
            nc.tensor.matmul(pwarm0[:, 0:C], Xb[:, 0, 0:P], Xb[:, w % NT, :],
                             start=True, stop=True)

        xn = [sb.tile([P, C], BF16, tag=f"xn{i}", name=f"xn{i}")
              for i in range(NT)]
        ptg = [ps.tile([P, N], BF16, tag=f"A{cc}", name=f"ptg{cc}")
               for cc in range(CT)]
        for i in range(NT):
            nc.vector.tensor_scalar(
                out=xn[i][:], in0=Xb[:, i, :], scalar1=RNO[:, i:i + 1],
                scalar2=None, op0=mult)
            for cc in range(CT):
                nc.tensor.transpose(ptg[cc][:, TS(i, P)], xn[i][:, TS(cc, P)],
                                    idb[:])
            if i % 2 == 1:
                # interleaved keep-warm matmuls (transpose-mode does not
                # count as PE activity for the HAM monitor)
                nc.tensor.matmul(pwarm0[:, 0:C], Xb[:, 0, 0:P],
                                 Xb[:, i, :], start=True, stop=True)
            if i % 4 == 3:
                # per-half copies so the j=0 A matmuls start one half early
                h = i // 4
                for cc in range(CT):
                    nc.vector.tensor_copy(xnT[:, cc, TS(h, 512)],
                                          ptg[cc][:, TS(h, 512)])
        if not diag_one:
            for cc in range(CT):
                nc.vector.tensor_scalar(
                    out=xnTs[:, cc, :], in0=xnT[:, cc, :],
                    scalar1=wdg[:, cc:cc + 1], scalar2=None, op0=mult)
        lhs_xnT = xnTs if not diag_one else xnT

        # ---------------- phase 2: A, exp, pyT accumulation ----------------
        # Software-pipelined emission: the PE queue is in-order, so pyT(k)
        # (which waits on exp(k)) is queued AFTER A(k+2) to avoid
        # head-of-line blocking the A matmul stream on the ScalarE exp chain.
        pyT = [ps.tile([P, N], F32, tag=f"PY{k}", name=f"pyT{k}")
               for k in range(CT)]

        def emit_a_block(k):
            # fp8 DoubleRow: both c-chunks contract in a single matmul
            pa_k = ps.tile([P, N], F32, tag=f"A{k % 2}", name=f"pa{k}")
            for j in range(2):
                nc.tensor.matmul(
                    pa_k[:, TS(j, 512)],
                    lhs_xnT[:, :, TS(k, P)],
                    xnT[:, :, TS(j, 512)],
                    start=True, stop=True, perf_mode=DR)
            return pa_k

        pa_tiles = {0: emit_a_block(0), 1: emit_a_block(1)}
        for k in range(NT):
            nc.scalar.activation(G8[:, k, :], pa_tiles.pop(k)[:], AF.Exp,
                                 scale=5.0, accum_out=SP[:, k:k + 1])
            # zero the diagonal block of G (it is exp(5*||xn_i||^2) ~ e^5)
            nc.vector.tensor_tensor(out=G8[:, k, TS(k, P)],
                                    in0=G8[:, k, TS(k, P)], in1=ng[:], op=mult)
            if k + 2 < NT:
                pa_tiles[k + 2] = emit_a_block(k + 2)
            if k % 2 == 1:
                # fp8 DoubleRow over the (k-1, k) pair of row blocks
                kp = k // 2
                for cc in range(CT):
                    for j in range(2):
                        nc.tensor.matmul(
                            pyT[cc][:, TS(j, 512)],
                            Xb[:, k - 1:k + 1, TS(cc, P)],
                            G8[:, k - 1:k + 1, TS(j, 512)],
                            start=(kp == 0), stop=(kp == NT // 2 - 1),
                            perf_mode=DR)

        # ---------------- phase 3: u = py@W^T, diff, LN, prelu -------------
        # -beta = 1/(e^5 - rowsum(G)); the e^5 removes the zeroed diagonal
        nc.vector.tensor_scalar(out=SPs[:], in0=SP[:], scalar1=-1.0,
                                scalar2=E5, op0=mult, op1=add)
        nc.vector.reciprocal(BET[:], SPs[:])

        # pyTs = cast of the raw c-major G@x out of PSUM (halves split
        # between ScalarE and DVE)
        for cc in range(CT):
            for h in range(2):
                if cc == 0:
                    nc.scalar.activation(spyT[cc][:, TS(h, 512)],
                                         pyT[cc][:, TS(h, 512)], AF.Copy)
                else:
                    nc.vector.tensor_copy(spyT[cc][:, TS(h, 512)],
                                          pyT[cc][:, TS(h, 512)])

        # diff tiles: pd[i] = sum_cc xt_i^T @ wtb_cc  (x@W^T part; these
        # matmuls only need DMA'd operands and keep the PE warm)
        pdA = ps.tile([P, N], F32, tag="A1", name="pdA")
        pdB = ps.tile([P, N], F32, tag="A0", name="pdB")
        pd = [pdA[:, TS(i, C)] for i in range(4)] + \
             [pdB[:, TS(i, C)] for i in range(4)]
        half = NT // 2
        for i in range(NT):
            for cc in range(CT):
                nc.tensor.matmul(pd[i], xtb[:, cc, TS(i, P)], wtb[:, cc, :],
                                 start=(i % 2 == 0 and cc == 0), stop=False)

        # u[i] = py_i @ W^T into the PSUM banks freed by pyT, then
        # us = -beta*u (per-partition scale, n-major), and pd += us via an
        # identity matmul so the subtraction costs no extra DVE pass.
        puA = ps.tile([P, N], F32, tag="PY0", name="puA")
        puB = ps.tile([P, N], F32, tag="PY1", name="puB")
        pu = [puA[:, TS(i, C)] for i in range(4)] + \
             [puB[:, TS(i, C)] for i in range(4)]
        ACT_STATS = {1, 3, 5} if ln_trivial else set()
        SU = sb.tile([P, NT], F32, tag="SU", name="SU")
        SQ = sb.tile([P, NT], F32, tag="SQ", name="SQ")
        us = [sb.tile([P, C], BF16, tag="us", bufs=4, name=f"us{i}")
              for i in range(NT)]

        def emit_stats(i):
            if i in ACT_STATS:
                ssc = sb.tile([P, C], F32, tag="ssc", bufs=2, name="ssc")
                nc.scalar.activation(ssc[:], pd[i], AF.Identity,
                                     accum_out=SU[:, i:i + 1])
                ssq = sb.tile([P, C], F32, tag="ssq", bufs=2, name="ssq")
                nc.scalar.activation(ssq[:], pd[i], AF.Square,
                                     accum_out=SQ[:, i:i + 1])
            else:
                nc.vector.bn_stats(BST[:, i, :], pd[i])
                nc.vector.bn_aggr(MV[:, i, :], BST[:, i, :])

        for p in range(NT // 2):
            for i in (2 * p, 2 * p + 1):
                for cc in range(CT):
                    nc.tensor.matmul(pu[i], spyT[cc][:, TS(i, P)],
                                     wtb[:, cc, :],
                                     start=(i % 2 == 0 and cc == 0),
                                     stop=(i % 2 == 1 and cc == CT - 1))
            for i in (2 * p, 2 * p + 1):
                nc.vector.tensor_scalar(out=us[i][:], in0=pu[i],
                                        scalar1=BET[:, i:i + 1],
                                        scalar2=None, op0=mult)
            for i in (2 * p, 2 * p + 1):
                last = (i % 2 == 1) and ln_trivial
                nc.tensor.matmul(pd[i], idb[:], us[i][:],
                                 start=False, stop=last)
                if not ln_trivial:
                    nc.tensor.matmul(pd[i], ones1[:], brow[:],
                                     start=False, stop=(i % 2 == 1))
            for i in (2 * p, 2 * p + 1):
                emit_stats(i)

        def emit_finish(h):
            # mu/var for the ScalarE-stats tiles of this half
            for i in sorted(ACT_STATS):
                if i // half != h:
                    continue
                nc.vector.tensor_scalar(out=MV[:, i, 0:1], in0=SU[:, i:i + 1],
                                        scalar1=1.0 / C, scalar2=None,
                                        op0=mult)
                nc.vector.scalar_tensor_tensor(
                    out=VPE[:, i:i + 1], in0=MV[:, i, 0:1], scalar=-1.0,
                    in1=MV[:, i, 0:1], op0=mult, op1=mult)
                nc.vector.scalar_tensor_tensor(
                    out=MV[:, i, 1:2], in0=SQ[:, i:i + 1], scalar=1.0 / C,
                    in1=VPE[:, i:i + 1], op0=mult, op1=add)
            hs = slice(h * half, (h + 1) * half)
            nc.vector.tensor_scalar(out=VPE[:, hs], in0=MV[:, hs, 1],
                                    scalar1=LN_EPS, scalar2=None, op0=add)
            _emit_rsqrt(nc, sb, RSTD[:, hs], VPE[:, hs],
                        ("rsA", "rsB", "rsC", "rsD"), half)
            nc.vector.scalar_tensor_tensor(out=NB[:, hs], in0=MV[:, hs, 0],
                                           scalar=-1.0, in1=RSTD[:, hs],
                                           op0=mult, op1=mult)
            for i in range(h * half, (h + 1) * half):
                if ln_trivial and use_prelu and i % 2 == 0:
                    nc.scalar.activation(Y[:, i, :], pd[i], AF.Prelu,
                                         bias=NB[:, i:i + 1],
                                         scale=RSTD[:, i:i + 1], alpha=LEAK)
                else:
                    t = sb.tile([P, C], BF16, tag="fin", bufs=2, name="fin")
                    nc.vector.tensor_scalar(
                        out=t[:], in0=pd[i], scalar1=RSTD[:, i:i + 1],
                        scalar2=NB[:, i:i + 1], op0=mult, op1=add)
                    if not ln_trivial:
                        u2 = sb.tile([P, C], F32, tag="fin2", bufs=2,
                                     name="fin2")
                        nc.vector.scalar_tensor_tensor(
                            out=u2[:], in0=t[:], scalar=1.0, in1=gbc[:],
                            op0=bypass, op1=mult)
                        nc.vector.scalar_tensor_tensor(
                            out=t[:], in0=u2[:], scalar=1.0, in1=bebc[:],
                            op0=bypass, op1=add)
                    nc.vector.scalar_tensor_tensor(
                        out=Y[:, i, :], in0=t[:], scalar=LEAK, in1=t[:],
                        op0=mult, op1=amax)
            nc.sync.dma_start(y_d[:, hs, :], Y[:, hs, :])

        emit_finish(0)
        emit_finish(1)


_PROGRAM_CACHE = {}
last_results = None


def _cfg():
    return {
        "prelu": bool(int(os.environ.get("KERNEL_PRELU", "1"))),
        "warm_mm": int(os.environ.get("KERNEL_WARM_MM", "8")),
        "xnt_dma": bool(int(os.environ.get("KERNEL_XNT_DMA", "1"))),
    }


def _get_program(diag_one=True, ln_trivial=True):
    cfg = _cfg()
    key = (diag_one, ln_trivial, tuple(sorted(cfg.items())))
    if key not in _PROGRAM_CACHE:
        _PROGRAM_CACHE[key] = _build_program(diag_one, ln_trivial, cfg)
    return _PROGRAM_CACHE[key]


def _prep_inputs(local_feat, W_adj, W_aff, b_aff, ln_gamma, ln_beta):
    x_full = np.asarray(local_feat, np.float32).reshape(B, N, C)
    diag = np.ascontiguousarray(np.diagonal(np.asarray(W_adj, np.float32)))
    diag_one = bool(np.all(diag == 1.0))
    g = np.asarray(ln_gamma, np.float32).ravel()
    be = np.asarray(ln_beta, np.float32).ravel()
    b = np.asarray(b_aff, np.float32).ravel()
    ln_trivial = bool(np.all(g == 1.0) and np.all(be == 0.0)
                      and np.all(b == 0.0))

    bf = ml_dtypes.bfloat16
    # wtb[cc, p, d] = W_aff[d, cc*128 + p]
    wt = np.ascontiguousarray(
        np.asarray(W_aff, np.float32).T.reshape(CT, P, C)).astype(bf)
    ng = (1.0 - np.eye(P, dtype=np.float32)).astype(bf)
    sel = np.broadcast_to(np.eye(NT, dtype=np.float32)[:, :, None],
                          (NT, NT, P)).astype(bf)
    aux = np.zeros((P, 8), np.float32)
    if not diag_one:
        aux[:, 2:4] = diag.reshape(CT, P).T

    f8 = ml_dtypes.float8_e4m3
    in_maps = []
    for bb in range(B):
        x = x_full[bb]
        xb = np.ascontiguousarray(
            x.reshape(NT, P, C).transpose(1, 0, 2)).astype(f8)
        xt = np.ascontiguousarray(
            x.T.reshape(CT, P, N)).astype(bf)
        in_maps.append({"xb": xb, "xt": xt, "wtb": wt, "ng": ng,
                        "aux": aux, "sel": sel})
    return in_maps, diag_one, ln_trivial


def kernel(local_feat, global_feat, pos, W_adj, W_aff, b_aff, ln_gamma,
           ln_beta, **_unused):
    global last_results
    in_maps, diag_one, ln_trivial = _prep_inputs(
        local_feat, W_adj, W_aff, b_aff, ln_gamma, ln_beta)
    nc = _get_program(diag_one, ln_trivial)
    trace = bool(int(os.environ.get("KERNEL_TRACE", "0")))
    res = run_bass_kernel_spmd(nc, in_maps, list(range(B)), trace=trace)
    last_results = res
    out = np.empty((B, N, C), np.float32)
    for bb in range(B):
        yb = np.asarray(res.results[bb]["y"]).astype(np.float32)  # [P, NT, C]
        out[bb] = yb.transpose(1, 0, 2).reshape(N, C)
    return out.reshape(B, T, NN, C)


# revision 55
# speedup vs baseline: 1.0251x; 1.0207x over previous
"""Trainium2 Bass kernel for nn_Diff_prop_18425409699925 (GNN message passing).

Math (per batch element b, x = local_feat[b] reshaped to [n=1024, c=256]):
  xn   = x / ||x||_row
  A    = xn @ xn^T                      (W_adj has unit diagonal; einsum uses
                                         only diag(W_adj))
  G    = exp(5*A), diagonal zeroed      (the reference's row-max shift cancels
                                         in the row-normalized mean)
  M    = (G @ x) / rowsum(G)
  diff = (x - M) @ W_aff^T + b_aff
  y    = LeakyReLU(LayerNorm(diff), 0.01)

Sharding: data-parallel over batch B=8, one element per NeuronCore, no
collectives.

Single fused pipeline per core (v2 restructure):
  phase 1: row norms via DVE square+accum, rsqrt via fp32 magic+Newton on DVE
           (keeps ScalarE down to ONE activation-table set), xn in bf16,
           xn^T built with DMA-engine xbar transposes (no PSUM, no DVE copies).
  phase 2: per row-block k: A row-block matmul (bf16), exp on ScalarE with
           row-sum accumulation, G diagonal zeroed, and the G @ x product
           accumulated immediately as pyT += X_k^T @ G[k] with X stationary
           (c-major output) so the PE streams 512-col matmuls with hidden
           weight loads and stays warm.
  phase 3: -1/rowsum broadcast via PE rank-1 matmuls, spyT = pyT * (-beta),
           diff[i] = x_i @ W^T - (py*beta)_i @ W^T accumulated in PSUM from
           c-major operands (no D transposes), LayerNorm stats via bn_stats/
           bn_aggr, and a single fused Prelu(scale=rstd, bias=-mu*rstd,
           alpha=0.01) finisher straight out of PSUM.

global_feat and pos are unused by the reference; accepted and ignored.
"""

import os
import sys

import numpy as np

for _p in ("/opt/trn_rl_repo",):
    if os.path.isdir(_p) and _p not in sys.path:
        sys.path.insert(0, _p)

import ml_dtypes

import concourse.bacc as bacc
import concourse.bass as bass
import concourse.tile as tile
from concourse import mybir
from concourse.bass_utils import run_bass_kernel_spmd

B, T, NN, C = 8, 16, 64, 256
N = T * NN            # 1024 nodes per batch element
P = 128               # partitions
NT = N // P           # 8 n-tiles
CT = C // P           # 2 c-tiles
F32 = mybir.dt.float32
BF16 = mybir.dt.bfloat16
F8 = mybir.dt.float8e4
I32 = mybir.dt.int32
TS = bass.ts
DR = mybir.MatmulPerfMode.DoubleRow

LN_EPS = 1e-5
LEAK = 0.01
E5 = 148.4131591025766          # exp(5): diagonal of exp(5*A) pre-zeroing
MAGIC = float(0x5F3759DF)       # fp32 rsqrt seed constant, as a float


def _emit_rsqrt(nc, sb, out, in_, tmp_tags, n):
    """out[:, :n] = 1/sqrt(in_[:, :n]) entirely on DVE (magic seed + 2 Newton
    steps). in_/out fp32 [P, n]. Avoids the ScalarE sqrt table set."""
    mult = mybir.AluOpType.mult
    add = mybir.AluOpType.add
    bypass = mybir.AluOpType.bypass
    f = sb.tile([P, n], F32, tag=tmp_tags[0], name="rsq_f")
    ii = sb.tile([P, n], I32, tag=tmp_tags[1], name="rsq_i")
    a = sb.tile([P, n], F32, tag=tmp_tags[2], name="rsq_a")
    c = sb.tile([P, n], F32, tag=tmp_tags[3], name="rsq_c")
    # seed: y = bitcast(MAGIC - (bitcast_int(s) >> 1)), done in fp32 arithmetic
    nc.vector.tensor_copy(f[:], in_.bitcast(I32))          # int -> float value
    nc.vector.tensor_scalar(out=f[:], in0=f[:], scalar1=-0.5, scalar2=MAGIC,
                            op0=mult, op1=add)
    nc.vector.tensor_copy(ii[:], f[:])                     # float -> int round
    y = ii.bitcast(F32)
    for _ in range(2):
        nc.vector.tensor_tensor(out=a[:], in0=y[:], in1=y[:], op=mult)
        nc.vector.scalar_tensor_tensor(out=a[:], in0=a[:], scalar=-0.5,
                                       in1=in_, op0=mult, op1=mult)
        nc.vector.tensor_scalar(out=c[:], in0=a[:], scalar1=1.5, scalar2=None,
                                op0=add)
        nc.vector.tensor_tensor(out=y[:], in0=y[:], in1=c[:], op=mult)
    nc.vector.tensor_copy(out, y[:])


def _build_program(diag_one, ln_trivial, cfg):
    nc = bacc.Bacc("TRN2", target_bir_lowering=False, debug=False)

    xb_d = nc.declare_dram_parameter("xb", [P, NT, C], F8, isOutput=False)
    xt_d = nc.declare_dram_parameter("xt", [CT, P, N], BF16, isOutput=False)
    wtb_d = nc.declare_dram_parameter("wtb", [CT, P, C], BF16, isOutput=False)
    ng_d = nc.declare_dram_parameter("ng", [P, P], BF16, isOutput=False)
    aux_d = nc.declare_dram_parameter("aux", [P, 8], F32, isOutput=False)
    sel_d = nc.declare_dram_parameter("sel", [NT, NT, P], BF16, isOutput=False)
    y_d = nc.declare_dram_parameter("y", [P, NT, C], BF16, isOutput=True)

    with tile.TileContext(nc) as tc:
        _emit(nc, tc, xb_d, xt_d, wtb_d, ng_d, aux_d, sel_d, y_d,
              diag_one, ln_trivial, cfg)
    nc.finalize()
    return nc


def _emit(nc, tc, xb_d, xt_d, wtb_d, ng_d, aux_d, sel_d, y_d,
          diag_one, ln_trivial, cfg):
    from contextlib import ExitStack

    mult = mybir.AluOpType.mult
    add = mybir.AluOpType.add
    bypass = mybir.AluOpType.bypass
    amax = mybir.AluOpType.max
    subtract = mybir.AluOpType.subtract
    AF = mybir.ActivationFunctionType

    use_prelu = cfg["prelu"]
    n_warm = cfg["warm_mm"]
    xnt_dma = cfg["xnt_dma"]

    with ExitStack() as ctx:
        sb = ctx.enter_context(tc.tile_pool(name="sb", bufs=1))
        ps = ctx.enter_context(tc.tile_pool(name="ps", bufs=1, space="PSUM"))

        # ---------------- SBUF tiles ----------------
        Xb = sb.tile([P, NT, C], F8, tag="Xb", name="Xb")
        xtb = sb.tile([P, CT, N], BF16, tag="xtb", name="xtb")
        wtb = sb.tile([P, CT, C], BF16, tag="wtb", name="wtb")
        xnT = sb.tile([P, CT, N], F8, tag="xnT", name="xnT")
        G8 = sb.tile([P, NT, N], F8, tag="G8", name="G8")
        spyT = [sb.tile([P, N], BF16, tag=f"spyT{k}", name=f"spyT{k}")
                for k in range(CT)]
        bbc = sb.tile([P, N], BF16, tag="bbc", name="bbc")
        ng = sb.tile([P, P], BF16, tag="ng", name="ng")
        aux = sb.tile([P, 8], F32, tag="aux", name="aux")
        sel = sb.tile([NT, NT, P], BF16, tag="sel", name="sel")
        Y = sb.tile([P, NT, C], BF16, tag="Y", name="Y")
        ones1 = sb.tile([1, P], F32, tag="ones1", name="ones1")

        SS = sb.tile([P, NT], F32, tag="SS", name="SS")
        RNO = sb.tile([P, NT], F32, tag="RNO", name="RNO")
        SP = sb.tile([P, NT], F32, tag="SP", name="SP")
        SPs = sb.tile([P, NT], F32, tag="SPs", name="SPs")
        BET = sb.tile([P, NT], F32, tag="BET", name="BET")
        BST = sb.tile([P, NT, 6], F32, tag="BST", name="BST")
        MV = sb.tile([P, NT, 2], F32, tag="MV", name="MV")
        VPE = sb.tile([P, NT], F32, tag="VPE", name="VPE")
        RSTD = sb.tile([P, NT], F32, tag="RSTD", name="RSTD")
        NB = sb.tile([P, NT], F32, tag="NB", name="NB")
        warm1 = sb.tile([P, 1], F32, tag="warm1", name="warm1")

        if not diag_one:
            wdg = sb.tile([P, CT], F32, tag="wdg", name="wdg")
            xnTs = sb.tile([P, CT, N], F8, tag="xnTs", name="xnTs")
        else:
            xnTs = None
        if not ln_trivial:
            gbc = sb.tile([P, C], F32, tag="gbc", name="gbc")
            bebc = sb.tile([P, C], F32, tag="bebc", name="bebc")
            brow = sb.tile([1, C], F32, tag="brow", name="brow")

        # PSUM tags A0/A1 (2 banks each) and PY0/PY1 (2 banks each); tiles are
        # created in first-write order below (generation order per tag).

        # ------- loads: x on sync (HWDGE), weights on gpsimd (SWDGE) so
        # neither the ScalarE nor the x-path queue is blocked ------------
        for q in range(4):
            nc.sync.dma_start(Xb[:, TS(q, 2), :], xb_d[:, TS(q, 2), :])
        nc.gpsimd.dma_start(ng[:], ng_d[:])
        nc.gpsimd.dma_start(aux[:], aux_d[:])
        nc.gpsimd.dma_start(sel[:], sel_d[:])
        nc.gpsimd.dma_start(wtb[:], wtb_d[:].rearrange("c p d -> p c d"))
        nc.gpsimd.dma_start(xtb[:], xt_d[:].rearrange("c p n -> p c n"))
        nc.vector.memset(ones1[:], 1.0)

        # Trigger the single ScalarE table load (exp set) immediately.
        nc.vector.memset(warm1[:], 0.0)
        wscr = sb.tile([P, 1], F32, tag="wscr", name="wscr")
        nc.scalar.activation(wscr[:], warm1[:], AF.Exp)

        if not diag_one:
            # aux[:, 2:4] carries diag(W_adj) rearranged [P, CT]
            nc.vector.tensor_copy(wdg[:], aux[:, 2:4])

        # ---------------- phase 1: norms, xn, xn^T ----------------
        for i in range(NT):
            # split the row-norm squares across DVE and ScalarE
            if i % 2 == 0:
                sqa = sb.tile([P, C], F32, tag="sqa", bufs=2, name="sqa")
                nc.scalar.activation(sqa[:], Xb[:, i, :], AF.Square,
                                     accum_out=SS[:, i:i + 1])
            else:
                sqs = sb.tile([P, C], F32, tag="sqs", bufs=2, name="sqs")
                nc.vector.scalar_tensor_tensor(
                    out=sqs[:], in0=Xb[:, i, :], scalar=1.0, in1=Xb[:, i, :],
                    op0=bypass, op1=mult, accum_out=SS[:, i:i + 1])
            if i == 3:
                _emit_rsqrt(nc, sb, RNO[:, 0:4], SS[:, 0:4],
                            ("rsA", "rsB", "rsC", "rsD"), 4)
        _emit_rsqrt(nc, sb, RNO[:, 4:NT], SS[:, 4:NT],
                    ("rsA", "rsB", "rsC", "rsD"), 4)

        # xn tiles, then PE transposes batched 8-per-PSUM-slot so the
        # PSUM->SBUF copies are one big [128,1024] CAST per c-chunk
        idb = sb.tile([P, P], BF16, tag="idb", name="idb")
        nc.vector.tensor_scalar(out=idb[:], in0=ng[:], scalar1=-1.0,
                                scalar2=1.0, op0=mult, op1=add)
        # Early PE warmup on the raw fp8 x tiles (ready as soon as the DMA
        # lands) so the HAM clock is at 2.4GHz before the transposes/A work.
        # Targets a PY-tag slot: its next generation (pyT) is written late.
        pwarm0 = ps.tile([P, 512], F32, tag="PY0", name="pwarm0")
        for w in range(n_warm):
            nc.tensor.matmul(pwarm0[:, 0:C], Xb[:, 0, 0:P], Xb[:, w % NT, :],
                             start=True, stop=True)

        xn = [sb.tile([P, C], BF16, tag=f"xn{i}", name=f"xn{i}")
              for i in range(NT)]
        ptg = [ps.tile([P, N], BF16, tag=f"A{cc}", name=f"ptg{cc}")
               for cc in range(CT)]
        # pa tiles for k=0,1 live in the PY-tag banks so their j=0 matmuls
        # can run while the second half of the transposes is still going
        # (the A-tag banks are only free once the ptg copies finish).
        pa_tiles = {
            0: ps.tile([P, N], F32, tag="PY0", name="pa0"),
            1: ps.tile([P, N], F32, tag="PY1", name="pa1"),
        }
        lhs_xnT = xnTs if not diag_one else xnT

        def emit_a_half(k, j):
            nc.tensor.matmul(
                pa_tiles[k][:, TS(j, 512)],
                lhs_xnT[:, :, TS(k, P)],
                xnT[:, :, TS(j, 512)],
                start=True, stop=True, perf_mode=DR)

        for i in range(NT):
            nc.vector.tensor_scalar(
                out=xn[i][:], in0=Xb[:, i, :], scalar1=RNO[:, i:i + 1],
                scalar2=None, op0=mult)
            for cc in range(CT):
                nc.tensor.transpose(ptg[cc][:, TS(i, P)], xn[i][:, TS(cc, P)],
                                    idb[:])
            if i < 4 and i % 2 == 1:
                # interleaved keep-warm matmuls (transpose-mode does not
                # count as PE activity for the HAM monitor)
                nc.tensor.matmul(pwarm0[:, 0:C], Xb[:, 0, 0:P],
                                 Xb[:, i, :], start=True, stop=True)
            if i % 4 == 3:
                # per-half copies so the j=0 A matmuls start one half early
                h = i // 4
                for cc in range(CT):
                    nc.vector.tensor_copy(xnT[:, cc, TS(h, 512)],
                                          ptg[cc][:, TS(h, 512)])
                if not diag_one:
                    for cc in range(CT):
                        nc.vector.tensor_scalar(
                            out=xnTs[:, cc, TS(h, 512)],
                            in0=xnT[:, cc, TS(h, 512)],
                            scalar1=wdg[:, cc:cc + 1], scalar2=None, op0=mult)
            # interleave the first A matmul halves with the h1 transposes
            if i == 4:
                emit_a_half(0, 0)
            elif i == 5:
                emit_a_half(1, 0)
        emit_a_half(0, 1)
        emit_a_half(1, 1)

        # ---------------- phase 2: A, exp, pyT accumulation ----------------
        # Software-pipelined emission: the PE queue is in-order, so pyT(k)
        # (which waits on exp(k)) is queued AFTER A(k+2) to avoid
        # head-of-line blocking the A matmul stream on the ScalarE exp chain.
        pyT = [ps.tile([P, N], F32, tag=f"PY{k}", name=f"pyT{k}")
               for k in range(CT)]

        def emit_a_block(k):
            # fp8 DoubleRow: both c-chunks contract in a single matmul
            pa_k = ps.tile([P, N], F32, tag=f"A{k % 2}", name=f"pa{k}")
            for j in range(2):
                nc.tensor.matmul(
                    pa_k[:, TS(j, 512)],
                    lhs_xnT[:, :, TS(k, P)],
                    xnT[:, :, TS(j, 512)],
                    start=True, stop=True, perf_mode=DR)
            return pa_k
        for k in range(NT):
            nc.scalar.activation(G8[:, k, :], pa_tiles.pop(k)[:], AF.Exp,
                                 scale=5.0, accum_out=SP[:, k:k + 1])
            # zero the diagonal block of G (it is exp(5*||xn_i||^2) ~ e^5)
            nc.vector.tensor_tensor(out=G8[:, k, TS(k, P)],
                                    in0=G8[:, k, TS(k, P)], in1=ng[:], op=mult)
            if k + 2 < NT:
                pa_tiles[k + 2] = emit_a_block(k + 2)
            if k % 2 == 1:
                # fp8 DoubleRow over the (k-1, k) pair of row blocks
                kp = k // 2
                for cc in range(CT):
                    for j in range(2):
                        nc.tensor.matmul(
                            pyT[cc][:, TS(j, 512)],
                            Xb[:, k - 1:k + 1, TS(cc, P)],
                            G8[:, k - 1:k + 1, TS(j, 512)],
                            start=(kp == 0), stop=(kp == NT // 2 - 1),
                            perf_mode=DR)

        # ---------------- phase 3: u = py@W^T, diff, LN, prelu -------------
        # -beta = 1/(e^5 - rowsum(G)); the e^5 removes the zeroed diagonal
        nc.vector.tensor_scalar(out=SPs[:], in0=SP[:], scalar1=-1.0,
                                scalar2=E5, op0=mult, op1=add)
        nc.vector.reciprocal(BET[:], SPs[:])

        # pyTs = cast of the raw c-major G@x out of PSUM (halves split
        # between ScalarE and DVE)
        for cc in range(CT):
            for h in range(2):
                if cc == 0:
                    nc.scalar.activation(spyT[cc][:, TS(h, 512)],
                                         pyT[cc][:, TS(h, 512)], AF.Copy)
                else:
                    nc.vector.tensor_copy(spyT[cc][:, TS(h, 512)],
                                          pyT[cc][:, TS(h, 512)])

        # diff tiles: pd[i] = sum_cc xt_i^T @ wtb_cc  (x@W^T part; these
        # matmuls only need DMA'd operands and keep the PE warm)
        pdA = ps.tile([P, N], F32, tag="A1", name="pdA")
        pdB = ps.tile([P, N], F32, tag="A0", name="pdB")
        pd = [pdA[:, TS(i, C)] for i in range(4)] + \
             [pdB[:, TS(i, C)] for i in range(4)]
        half = NT // 2
        for i in range(NT):
            for cc in range(CT):
                nc.tensor.matmul(pd[i], xtb[:, cc, TS(i, P)], wtb[:, cc, :],
                                 start=(i % 2 == 0 and cc == 0), stop=False)

        # u[i] = py_i @ W^T into the PSUM banks freed by pyT, then
        # us = -beta*u (per-partition scale, n-major), and pd += us via an
        # identity matmul so the subtraction costs no extra DVE pass.
        puA = ps.tile([P, N], F32, tag="PY0", name="puA")
        puB = ps.tile([P, N], F32, tag="PY1", name="puB")
        pu = [puA[:, TS(i, C)] for i in range(4)] + \
             [puB[:, TS(i, C)] for i in range(4)]
        ACT_STATS = {1, 3, 5} if ln_trivial else set()
        SU = sb.tile([P, NT], F32, tag="SU", name="SU")
        SQ = sb.tile([P, NT], F32, tag="SQ", name="SQ")
        us = [sb.tile([P, C], BF16, tag="us", bufs=4, name=f"us{i}")
              for i in range(NT)]

        def emit_stats(i):
            if i in ACT_STATS:
                ssc = sb.tile([P, C], F32, tag="ssc", bufs=2, name="ssc")
                nc.scalar.activation(ssc[:], pd[i], AF.Identity,
                                     accum_out=SU[:, i:i + 1])
                ssq = sb.tile([P, C], F32, tag="ssq", bufs=2, name="ssq")
                nc.scalar.activation(ssq[:], pd[i], AF.Square,
                                     accum_out=SQ[:, i:i + 1])
            else:
                nc.vector.bn_stats(BST[:, i, :], pd[i])
                nc.vector.bn_aggr(MV[:, i, :], BST[:, i, :])

        for p in range(NT // 2):
            for i in (2 * p, 2 * p + 1):
                for cc in range(CT):
                    nc.tensor.matmul(pu[i], spyT[cc][:, TS(i, P)],
                                     wtb[:, cc, :],
                                     start=(i % 2 == 0 and cc == 0),
                                     stop=(i % 2 == 1 and cc == CT - 1))
            for i in (2 * p, 2 * p + 1):
                nc.vector.tensor_scalar(out=us[i][:], in0=pu[i],
                                        scalar1=BET[:, i:i + 1],
                                        scalar2=None, op0=mult)
            for i in (2 * p, 2 * p + 1):
                last = (i % 2 == 1) and ln_trivial
                nc.tensor.matmul(pd[i], idb[:], us[i][:],
                                 start=False, stop=last)
                if not ln_trivial:
                    nc.tensor.matmul(pd[i], ones1[:], brow[:],
                                     start=False, stop=(i % 2 == 1))
            for i in (2 * p, 2 * p + 1):
                emit_stats(i)

        def emit_finish(h):
            # mu/var for the ScalarE-stats tiles of this half
            for i in sorted(ACT_STATS):
                if i // half != h:
                    continue
                nc.vector.tensor_scalar(out=MV[:, i, 0:1], in0=SU[:, i:i + 1],
                                        scalar1=1.0 / C, scalar2=None,
                                        op0=mult)
                nc.vector.scalar_tensor_tensor(
                    out=VPE[:, i:i + 1], in0=MV[:, i, 0:1], scalar=-1.0,
                    in1=MV[:, i, 0:1], op0=mult, op1=mult)
                nc.vector.scalar_tensor_tensor(
                    out=MV[:, i, 1:2], in0=SQ[:, i:i + 1], scalar=1.0 / C,
                    in1=VPE[:, i:i + 1], op0=mult, op1=add)
            hs = slice(h * half, (h + 1) * half)
            nc.vector.tensor_scalar(out=VPE[:, hs], in0=MV[:, hs, 1],
                                    scalar1=LN_EPS, scalar2=None, op0=add)
            _emit_rsqrt(nc, sb, RSTD[:, hs], VPE[:, hs],
                        ("rsA", "rsB", "rsC", "rsD"), half)
            nc.vector.scalar_tensor_tensor(out=NB[:, hs], in0=MV[:, hs, 0],
                                           scalar=-1.0, in1=RSTD[:, hs],
                                           op0=mult, op1=mult)
            for i in range(h * half, (h + 1) * half):
                if ln_trivial and use_prelu and i % 2 == 0:
                    nc.scalar.activation(Y[:, i, :], pd[i], AF.Prelu,
                                         bias=NB[:, i:i + 1],
                                         scale=RSTD[:, i:i + 1], alpha=LEAK)
                else:
                    t = sb.tile([P, C], BF16, tag="fin", bufs=2, name="fin")
                    nc.vector.tensor_scalar(
                        out=t[:], in0=pd[i], scalar1=RSTD[:, i:i + 1],
                        scalar2=NB[:, i:i + 1], op0=mult, op1=add)
                    if not ln_trivial:
                        u2 = sb.tile([P, C], F32, tag="fin2", bufs=2,
                                     name="fin2")
                        nc.vector.scalar_tensor_tensor(
                            out=u2[:], in0=t[:], scalar=1.0, in1=gbc[:],
                            op0=bypass, op1=mult)
                        nc.vector.scalar_tensor_tensor(
                            out=t[:], in0=u2[:], scalar=1.0, in1=bebc[:],
                            op0=bypass, op1=add)
                    nc.vector.scalar_tensor_tensor(
                        out=Y[:, i, :], in0=t[:], scalar=LEAK, in1=t[:],
                        op0=mult, op1=amax)
                if i % 2 == 1:
                    # ship each finished pair immediately to shorten the
                    # final write's completion tail
                    nc.sync.dma_start(y_d[:, i - 1:i + 1, :],
                                      Y[:, i - 1:i + 1, :])

        emit_finish(0)
        emit_finish(1)


_PROGRAM_CACHE = {}
last_results = None


def _cfg():
    return {
        "prelu": bool(int(os.environ.get("KERNEL_PRELU", "1"))),
        "warm_mm": int(os.environ.get("KERNEL_WARM_MM", "8")),
        "xnt_dma": bool(int(os.environ.get("KERNEL_XNT_DMA", "1"))),
    }


def _get_program(diag_one=True, ln_trivial=True):
    cfg = _cfg()
    key = (diag_one, ln_trivial, tuple(sorted(cfg.items())))
    if key not in _PROGRAM_CACHE:
        _PROGRAM_CACHE[key] = _build_program(diag_one, ln_trivial, cfg)
    return _PROGRAM_CACHE[key]


def _prep_inputs(local_feat, W_adj, W_aff, b_aff, ln_gamma, ln_beta):
    x_full = np.asarray(local_feat, np.float32).reshape(B, N, C)
    diag = np.ascontiguousarray(np.diagonal(np.asarray(W_adj, np.float32)))
    diag_one = bool(np.all(diag == 1.0))
    g = np.asarray(ln_gamma, np.float32).ravel()
    be = np.asarray(ln_beta, np.float32).ravel()
    b = np.asarray(b_aff, np.float32).ravel()
    ln_trivial = bool(np.all(g == 1.0) and np.all(be == 0.0)
                      and np.all(b == 0.0))

    bf = ml_dtypes.bfloat16
    # wtb[cc, p, d] = W_aff[d, cc*128 + p]
    wt = np.ascontiguousarray(
        np.asarray(W_aff, np.float32).T.reshape(CT, P, C)).astype(bf)
    ng = (1.0 - np.eye(P, dtype=np.float32)).astype(bf)
    sel = np.broadcast_to(np.eye(NT, dtype=np.float32)[:, :, None],
                          (NT, NT, P)).astype(bf)
    aux = np.zeros((P, 8), np.float32)
    if not diag_one:
        aux[:, 2:4] = diag.reshape(CT, P).T

    f8 = ml_dtypes.float8_e4m3
    in_maps = []
    for bb in range(B):
        x = x_full[bb]
        xb = np.ascontiguousarray(
            x.reshape(NT, P, C).transpose(1, 0, 2)).astype(f8)
        xt = np.ascontiguousarray(
            x.T.reshape(CT, P, N)).astype(bf)
        in_maps.append({"xb": xb, "xt": xt, "wtb": wt, "ng": ng,
                        "aux": aux, "sel": sel})
    return in_maps, diag_one, ln_trivial


def kernel(local_feat, global_feat, pos, W_adj, W_aff, b_aff, ln_gamma,
           ln_beta, **_unused):
    global last_results
    in_maps, diag_one, ln_trivial = _prep_inputs(
        local_feat, W_adj, W_aff, b_aff, ln_gamma, ln_beta)
    nc = _get_program(diag_one, ln_trivial)
    trace = bool(int(os.environ.get("KERNEL_TRACE", "0")))
    res = run_bass_kernel_spmd(nc, in_maps, list(range(B)), trace=trace)
    last_results = res
    out = np.empty((B, N, C), np.float32)
    for bb in range(B):
        yb = np.asarray(res.results[bb]["y"]).astype(np.float32)  # [P, NT, C]
        out[bb] = yb.transpose(1, 0, 2).reshape(N, C)
    return out.reshape(B, T, NN, C)
